# revision 40
# baseline (speedup 1.0000x reference)
"""Nystromformer sparse attention on 8 Trainium2 NeuronCores.

Sharding: core = bi*4 + g handles batch bi (of 2) and heads {2g, 2g+1}
(of 8). All landmark/pinv work is per-(b,h); the final to_out matmul is
computed per-core against the matching W_out row-slice and the partial
(1024, 512) outputs are summed on the host (4 partials per batch).

Key algorithmic reformulation (exact algebra): the Moore-Penrose
iteration on attn2 (1024x256) runs entirely in 256x256 space:
z_k = W_k @ attn2^T with W symmetric, so each iteration is 4 256^3
matmuls instead of two 1024^3 ones. The global max(col)*max(row) init
scale is reproduced exactly: max(col)=1 (softmax rows), and max(row) is
an in-kernel AllReduce(max) across all 8 cores. The iteration starts
from W0 = (1/c) I and runs on the raw G (same trajectory as the
reference's z0 = attn2^T / c), so only W0 waits on the collective.

Precision: ALL wide matmuls run in float32r (HW probe: f32r rounds
operands to 12-bit mantissa; an emulator of the full pipeline predicts
~5e-3 max rel err for all-f32r vs ~4.5e-3 for the old fp32-mixed
schedule -- error is dominated by the always-f32r early Newton-Schulz
iterations, so fp32 elsewhere bought nothing at 4x PE cost).

Schedule (every engine queue is in-order, so emission order must track
readiness): the sim3->exp->av stream starts as soon as the first kT
slice lands (~6us) on a dedicated 3-buf PSUM tag; the landmark path
(s1, column sums via r1r-weighted matmuls of the unnormalized exp, and
the AllReduce) is emitted between av-groups 0 and 1 so the collective
launches ~35us; the two heads' Newton-Schulz chains are emitted
iteration-interleaved and reuse the projection-phase PSUM bufs; the h0
tail (transpose/normalize/t1) is emitted before the iterations, the h1
tail after, so no in-order queue head-blocks. Column-sum trick:
cs = sum_i E1[i,m]/r1[i] as an accumulated matmul with lhsT = 1/r1,
removing the A-normalization from the collective's critical path.
"""

import json
import sys

for _p in ("/opt/trn_rl_repo", "/root/.axon_site/_ro/trn_rl_repo"):
    if _p not in sys.path:
        sys.path.append(_p)

import ml_dtypes
import numpy as np

import concourse.bass as bass
import concourse.mybir as mybir
import concourse.tile as tile
from concourse.bass_utils import run_bass_kernel_spmd

F32 = mybir.dt.float32
F32R = mybir.dt.float32r
BF16 = mybir.dt.bfloat16
AX = mybir.AxisListType
ALU = mybir.AluOpType
EXP = mybir.ActivationFunctionType.Exp

P = 128
DIM = 512
CH = 4  # contraction chunks of 128 over DIM
N = 4096
NS = 8  # 512-wide n slices
NJ = 32  # 128-wide j tiles
NQ = 1024
NIH = 2  # 512-wide i halves
NIT = 8  # 128-wide i tiles
M = 256
MT = 2  # 128-wide m tiles
DH = 64
ITERS = 6
NCORES = 8


def _split_multi_waits(bir_json_bytes: bytes) -> bytes:
    """Walrus in this container accepts one sync wait per instruction."""
    bir = json.loads(bir_json_bytes)
    for fn in bir.get("functions", []):
        for blk in fn.get("blocks", []):
            out = []
            for inst in blk.get("instructions", []):
                si = inst.get("sync_info")
                waits = (si or {}).get("on_wait") or []
                if len(waits) > 1:
                    for i, w in enumerate(waits[:-1]):
                        out.append(
                            {
                                "name": f"{inst['name']}-wsplit{i}",
                                "opcode": "NoOp",
                                "engine": inst["engine"],
                                "ins": [],
                                "outs": [],
                                "sync_info": {"on_wait": [w], "on_update": []},
                            }
                        )
                    si["on_wait"] = [waits[-1]]
                out.append(inst)
            blk["instructions"] = out
    return json.dumps(bir).encode()


def _install_wait_split_hook(nc):
    orig = nc.to_json_bytes

    def patched():
        return _split_multi_waits(orig())

    nc.to_json_bytes = patched


def _diag_ones(nc, ap, k):
    nc.gpsimd.affine_select(
        out=ap,
        in_=ap,
        compare_op=ALU.not_equal,
        fill=1.0,
        base=0,
        pattern=[[-1, k]],
        channel_multiplier=1,
    )


def build_kernel() -> bass.Bass:
    nc = bass.Bass("TRN2", num_devices=NCORES)

    xT_d = nc.dram_tensor("xT", [DIM, N], BF16, kind="ExternalInput")
    qT_d = nc.dram_tensor("qT_in", [DIM, NQ], BF16, kind="ExternalInput")
    wq_d = nc.dram_tensor("wq", [DIM, P], BF16, kind="ExternalInput")
    wk_d = nc.dram_tensor("wk", [DIM, P], BF16, kind="ExternalInput")
    wv_d = nc.dram_tensor("wv", [DIM, P], BF16, kind="ExternalInput")
    wout_d = nc.dram_tensor("wout", [P, DIM], F32R, kind="ExternalInput")
    y_d = nc.dram_tensor("y", [NQ, DIM], F32, kind="ExternalOutput")

    xr = xT_d.rearrange("(c p) n -> c p n", p=P)
    qr = qT_d.rearrange("(c p) n -> c p n", p=P)
    yr = y_d.rearrange("(t p) f -> t p f", p=P)

    with tile.TileContext(nc) as tc:
        with (
            tc.tile_pool(name="const", bufs=1) as cpool,
            tc.tile_pool(name="work", bufs=3) as wpool,
            tc.tile_pool(name="iter", bufs=2) as ipool,
            tc.tile_pool(name="ps", bufs=1, space="PSUM") as ps,
            tc.tile_pool(name="dram", bufs=1, space="DRAM") as dpool,
        ):
            # ------------- weights / constants (q-path DMAs on ACT queue) --
            wq_sb = cpool.tile([P, CH, P], BF16, tag="wq", name="wq")
            wk_sb = cpool.tile([P, CH, P], BF16, tag="wk", name="wk")
            wv_sb = cpool.tile([P, CH, P], BF16, tag="wv", name="wv")
            wout_sb = cpool.tile([P, DIM], F32R, tag="wout", name="wout")
            nc.sync.dma_start(wq_sb[:], wq_d.rearrange("(c p) m -> p c m", p=P))
            nc.sync.dma_start(wk_sb[:], wk_d.rearrange("(c p) m -> p c m", p=P))
            nc.sync.dma_start(wv_sb[:], wv_d.rearrange("(c p) m -> p c m", p=P))

            ones_col = cpool.tile([P, 1], F32, tag="ones", name="ones")
            nc.vector.memset(ones_col[:], 1.0)
            ident = cpool.tile([P, P], F32, tag="ident", name="ident")
            nc.vector.memset(ident[:], 0.0)
            _diag_ones(nc, ident[:], P)
            ident2 = cpool.tile([P, MT, M], F32, tag="ident2", name="ident2")
            nc.vector.memset(ident2[:], 0.0)
            _diag_ones(nc, ident2[:, 0, 0:P], P)
            _diag_ones(nc, ident2[:, 1, P:M], P)

            # ---------------- q projection (f32r) -------------------------
            qT_sb = cpool.tile([P, NQ], F32R, tag="qT", name="qT")
            q_pss = [
                ps.tile([P, 512], F32, tag="kvit", bufs=3, name="qps")
                for _ in range(NIH)
            ]
            for c in range(CH):
                qb = wpool.tile([P, NQ], BF16, tag="qb", bufs=3, name="qb")
                nc.sync.dma_start(qb[:], qr[c])
                for ih in range(NIH):
                    nc.tensor.matmul(
                        q_pss[ih][:],
                        wq_sb[:, c, :],
                        qb[:, ih * 512 : (ih + 1) * 512],
                        start=(c == 0),
                        stop=(c == CH - 1),
                    )
            for ih in range(NIH):
                nc.vector.tensor_copy(
                    qT_sb[:, ih * 512 : (ih + 1) * 512], q_pss[ih][:]
                )
            nc.sync.dma_start(wout_sb[:], wout_d[:])

            # ------- k/v projections + incremental landmark pool + vtr ----
            # vaug layout [P(j), NJ, 130]: 0:64 h0 v, 64 ones, 65:129 h1 v,
            # 129 ones
            kT_sb = cpool.tile([P, N], F32R, tag="kT", name="kT")
            vT_sb = cpool.tile([P, N], F32R, tag="vT", name="vT")
            klT_sb = cpool.tile([P, M], F32R, tag="klT", name="klT")
            vaug_sb = cpool.tile([P, NJ, 130], BF16, tag="vaug", name="vaug")
            for h in range(2):
                nc.vector.tensor_copy(
                    vaug_sb[:, :, h * 65 + DH : h * 65 + DH + 1],
                    ones_col[:, 0:1, None].to_broadcast((P, NJ, 1)),
                )
            for half in range(2):
                hsl = slice(half * 2048, (half + 1) * 2048)
                xbs = []
                for c in range(CH):
                    xb = wpool.tile([P, 2048], BF16, tag="xb", bufs=5, name="xb")
                    nc.sync.dma_start(xb[:], xr[c][:, hsl])
                    xbs.append(xb)
                for nsl in range(4):
                    ns = half * 4 + nsl
                    sl = slice(ns * 512, (ns + 1) * 512)
                    ssl = slice(nsl * 512, (nsl + 1) * 512)
                    k_ps = ps.tile([P, 512], F32, tag="kvit", bufs=3, name="kps")
                    v_ps = ps.tile([P, 512], F32, tag="kvit", bufs=3, name="vps")
                    for c in range(CH):
                        nc.tensor.matmul(
                            k_ps[:],
                            wk_sb[:, c, :],
                            xbs[c][:, ssl],
                            start=(c == 0),
                            stop=(c == CH - 1),
                        )
                        nc.tensor.matmul(
                            v_ps[:],
                            wv_sb[:, c, :],
                            xbs[c][:, ssl],
                            start=(c == 0),
                            stop=(c == CH - 1),
                        )
                    nc.vector.tensor_copy(kT_sb[:, sl], k_ps[:])
                    nc.vector.tensor_copy(vT_sb[:, sl], v_ps[:])
                    with nc.allow_low_precision(
                        reason="f32r tiles carry fp32 bits"
                    ):
                        nc.vector.reduce_sum(
                            klT_sb[:, ns * 32 : (ns + 1) * 32],
                            kT_sb[:, sl]
                            .bitcast(F32)
                            .rearrange("p (m l) -> p m l", l=16),
                            axis=AX.X,
                        )
                    for sub in range(4):
                        jt = ns * 4 + sub
                        tr_ps = ps.tile([P, P], F32, tag="small", bufs=2, name="vtrps")
                        nc.tensor.transpose(
                            tr_ps[:],
                            vT_sb[:, jt * P : (jt + 1) * P].bitcast(F32),
                            ident[:],
                        )
                        nc.vector.tensor_copy(
                            vaug_sb[:, jt, :].rearrange("p (g d) -> p g d", g=2)[
                                :, :, 0:DH
                            ],
                            tr_ps[:].rearrange("p (g d) -> p g d", g=2),
                        )

            # helpers for the s3 -> exp -> av stream -----------------------
            avu_sb = [
                cpool.tile([P, NIH, 512], F32, tag=f"avu{h}", name=f"avu{h}")
                for h in range(2)
            ]

            def av_group(h, ih):
                hs = slice(h * DH, (h + 1) * DH)
                av_ps = ps.tile([DH + 1, 512], F32, tag="hold", bufs=1, name="avps")
                for jt in range(NJ):
                    s3_ps = ps.tile([P, 512], F32, tag="s3", bufs=2, name="s3ps")
                    nc.tensor.matmul(
                        s3_ps[:],
                        kT_sb[hs, jt * P : (jt + 1) * P],
                        qT_sb[hs, ih * 512 : (ih + 1) * 512],
                        start=True,
                        stop=True,
                    )
                    e3 = wpool.tile([P, 512], BF16, tag="e3", name="e3")
                    nc.scalar.activation(e3[:], s3_ps[:], EXP)
                    nc.tensor.matmul(
                        av_ps[:],
                        vaug_sb[:, jt, h * 65 : h * 65 + 65],
                        e3[:],
                        start=(jt == 0),
                        stop=(jt == NJ - 1),
                    )
                # ACT copies the accumulator out: keeps DVE queue untangled
                nc.scalar.copy(avu_sb[h][: DH + 1, ih, :], av_ps[:])

            at_sb = [
                cpool.tile([P, NIT, DH + 1], F32, tag=f"at{h}", name=f"at{h}")
                for h in range(2)
            ]

            def av_transposes(h, ih):
                for isub in range(4):
                    it = ih * 4 + isub
                    at_ps = ps.tile([P, DH + 1], F32, tag="small", bufs=2, name="avtps")
                    nc.tensor.transpose(
                        at_ps[:],
                        avu_sb[h][: DH + 1, ih, isub * P : (isub + 1) * P],
                        ident[: DH + 1, : DH + 1],
                    )
                    nc.vector.tensor_copy(at_sb[h][:, it, :], at_ps[:])

            av_sb = [
                cpool.tile([P, NIT, DH], F32R, tag=f"av{h}", name=f"av{h}")
                for h in range(2)
            ]
            r3c_sb = [
                cpool.tile([P, NIT], F32, tag=f"r3c{h}", name=f"r3c{h}")
                for h in range(2)
            ]
            r3r_sb = [
                cpool.tile([P, NIT], F32, tag=f"r3r{h}", name=f"r3r{h}")
                for h in range(2)
            ]

            def av_normalize(h):
                for it in range(NIT):
                    nc.vector.tensor_copy(
                        r3c_sb[h][:, it : it + 1], at_sb[h][:, it, DH : DH + 1]
                    )
                nc.vector.reciprocal(r3r_sb[h][:], r3c_sb[h][:])
                for it in range(NIT):
                    nc.vector.tensor_scalar_mul(
                        av_sb[h][:, it, :],
                        at_sb[h][:, it, 0:DH],
                        r3r_sb[h][:, it : it + 1],
                    )

            A_sb = [
                cpool.tile([P, NIT, M], F32R, tag=f"A{h}", name=f"A{h}") for h in range(2)
            ]

            def t1_of(h):
                t1_sb = wpool.tile([P, MT, DH], F32R, tag=f"t1_{h}", bufs=1,
                                   name=f"t1_{h}")
                for mc in range(MT):
                    t1_ps = ps.tile([P, DH], F32, tag="small", bufs=2, name="t1ps")
                    for it in range(NIT):
                        nc.tensor.matmul(
                            t1_ps[:],
                            A_sb[h][:, it, mc * P : (mc + 1) * P],
                            av_sb[h][:, it, :],
                            start=(it == 0),
                            stop=(it == NIT - 1),
                        )
                    nc.vector.tensor_copy(t1_sb[:, mc, :], t1_ps[:])
                return t1_sb

            # ---------------- landmark path: s1 -> cs -> AllReduce --------
            r1s_sb = [
                cpool.tile([P, NIT], F32, tag=f"r1s{h}", name=f"r1s{h}")
                for h in range(2)
            ]
            r1r_sb = [
                cpool.tile([P, NIT], F32R, tag=f"r1r{h}", name=f"r1r{h}")
                for h in range(2)
            ]
            cmax2_sb = wpool.tile([1, 2], F32, tag="cmax2", name="cmax2")
            for h in range(2):
                hs = slice(h * DH, (h + 1) * DH)
                for it in range(NIT):
                    s1_ps = ps.tile([P, M], F32, tag="small", bufs=2, name="s1ps")
                    nc.tensor.matmul(
                        s1_ps[:],
                        qT_sb[hs, it * P : (it + 1) * P],
                        klT_sb[hs, :],
                        start=True,
                        stop=True,
                    )
                    # A_sb holds UNNORMALIZED exp until the cs matmuls read it
                    nc.scalar.activation(
                        A_sb[h][:, it, :],
                        s1_ps[:],
                        EXP,
                        accum_out=r1s_sb[h][:, it : it + 1],
                    )
                with nc.allow_low_precision(reason="f32r carries fp32 bits"):
                    nc.vector.reciprocal(r1r_sb[h][:], r1s_sb[h][:])
                # cs[m] = sum_i E1[i,m]/r1[i]: lhsT = 1/r1 column per i-tile
                cs_ps = ps.tile([1, M], F32, tag="small", bufs=2, name="csps")
                for it in range(NIT):
                    nc.tensor.matmul(
                        cs_ps[:],
                        r1r_sb[h][:, it : it + 1],
                        A_sb[h][:, it, :],
                        start=(it == 0),
                        stop=(it == NIT - 1),
                    )
                nc.vector.reduce_max(cmax2_sb[0:1, h : h + 1], cs_ps[:], axis=AX.X)
            cmax_sb = wpool.tile([1, 1], F32, tag="cmax", name="cmax")
            nc.vector.reduce_max(cmax_sb[:], cmax2_sb[:], axis=AX.X)
            bounce_sb = wpool.tile([1, 16], F32, tag="bounce", name="bounce")
            nc.vector.tensor_copy(bounce_sb[:], cmax_sb[0:1, 0:1].to_broadcast((1, 16)))
            cin_dram = dpool.tile([1, 16], F32)
            cout_dram = dpool.tile([1, 16 * NCORES], F32)
            nc.sync.dma_start(cin_dram[:], bounce_sb[:])
            nc.gpsimd.collective_compute(
                "AllGather",
                ALU.bypass,
                replica_groups=[list(range(NCORES))],
                ins=[cin_dram.opt()],
                outs=[cout_dram.opt()],
            )
            c128_sb = wpool.tile([P, 16 * NCORES], F32, tag="c128", name="c128")
            nc.sync.dma_start(
                c128_sb[:], cout_dram[0:1, :].to_broadcast((P, 16 * NCORES))
            )

            # A normalize (off critical path) + G = A^T A ------------------
            for h in range(2):
                for it in range(NIT):
                    nc.vector.tensor_scalar_mul(
                        A_sb[h][:, it, :],
                        A_sb[h][:, it, :].bitcast(F32),
                        r1r_sb[h][:, it : it + 1].bitcast(F32),
                    )
            # E1T = transpose of NORMALIZED A: the 1/r1 row scaling rides
            # along, so the final E1 @ t2 needs no normalization.
            E1T_sb = [
                cpool.tile([P, MT, NQ], F32R, tag=f"E1T{h}", name=f"E1T{h}")
                for h in range(2)
            ]
            for h in range(2):
                for it in range(NIT):
                    et_ps = ps.tile([P, P], F32, tag="small", bufs=2, name="etps")
                    for mt in range(MT):
                        nc.tensor.transpose(
                            et_ps[:],
                            A_sb[h][:, it, mt * P : (mt + 1) * P].bitcast(F32),
                            ident[:],
                        )
                        nc.vector.tensor_copy(
                            E1T_sb[h][:, mt, it * P : (it + 1) * P], et_ps[:]
                        )
            G_sb = [
                cpool.tile([P, MT, M], F32R, tag=f"G{h}", name=f"G{h}") for h in range(2)
            ]
            for h in range(2):
                for mc in range(MT):
                    g_ps = ps.tile([P, M], F32, tag="small", bufs=2, name="gps")
                    for it in range(NIT):
                        nc.tensor.matmul(
                            g_ps[:],
                            A_sb[h][:, it, mc * P : (mc + 1) * P],
                            A_sb[h][:, it, :],
                            start=(it == 0),
                            stop=(it == NIT - 1),
                        )
                    nc.vector.tensor_copy(G_sb[h][:, mc, :], g_ps[:])

            # ---------------- av group 0 (h0, ih0) ------------------------
            av_group(0, 0)
            av_transposes(0, 0)

            # ---------------- av group 1 + h0 tail ------------------------
            av_group(0, 1)
            av_transposes(0, 1)
            av_normalize(0)
            t1_h0 = t1_of(0)

            # ---------------- av groups 2, 3 (h1) -------------------------
            av_group(1, 0)
            av_group(1, 1)

            # ---------------- rc, W0, Newton-Schulz (head-interleaved) ----
            cmax128_sb = wpool.tile([P, 1], F32, tag="cmax128", name="cmax128")
            nc.vector.reduce_max(cmax128_sb[:], c128_sb[:], axis=AX.X)
            rc_sb = cpool.tile([P, 1], F32, tag="rc", name="rc")
            nc.vector.reciprocal(rc_sb[:], cmax128_sb[:])
            W_cur = []
            for h in range(2):
                W0 = ipool.tile([P, MT, M], F32R, tag=f"W{h}", bufs=1, name=f"W{h}")
                nc.vector.tensor_scalar_mul(W0[:], ident2[:], rc_sb[:, 0:1])
                W_cur.append(W0)
            for i in range(ITERS - 1):
                V_sb, B1s, B2s = {}, {}, {}
                for h in range(2):
                    V_sb[h] = ipool.tile(
                        [P, MT, M], F32R, tag=f"V{h}", bufs=1, name=f"V{h}{i}"
                    )
                    v_ps2 = ps.tile([P, MT, M], F32, tag="kvit", bufs=3, name="iterps")
                    for a in range(MT):
                        for t in range(MT):
                            nc.tensor.matmul(
                                v_ps2[:, a, :],
                                G_sb[h][:, t, a * P : (a + 1) * P],
                                W_cur[h][:, t, :],
                                start=(t == 0),
                                stop=(t == MT - 1),
                            )
                    nc.vector.tensor_copy(V_sb[h][:], v_ps2[:])
                for h in range(2):
                    B1s[h] = ipool.tile(
                        [P, MT, M], F32R, tag=f"B1{h}", bufs=1, name=f"B1{h}{i}"
                    )
                    b1_ps = ps.tile([P, MT, M], F32, tag="kvit", bufs=3, name="iterps")
                    for a in range(MT):
                        for t in range(MT):
                            nc.tensor.matmul(
                                b1_ps[:, a, :],
                                W_cur[h][:, t, a * P : (a + 1) * P],
                                V_sb[h][:, t, :],
                                start=(t == 0),
                                stop=(t == MT - 1),
                            )
                    nc.vector.tensor_scalar_mul(B1s[h][:], b1_ps[:], -3.75)
                for h in range(2):
                    # off-chain: tmp = 3.25*W + B1s (Pool, SBUF-only)
                    tmp = wpool.tile(
                        [P, MT, M], F32, tag=f"wtmp{h}", bufs=1, name=f"wtmp{h}"
                    )
                    nc.vector.scalar_tensor_tensor(
                        tmp[:],
                        W_cur[h][:].bitcast(F32),
                        3.25,
                        B1s[h][:].bitcast(F32),
                        ALU.mult,
                        ALU.add,
                    )
                    B2s[h] = ipool.tile(
                        [P, MT, M], F32R, tag=f"B2{h}", bufs=1, name=f"B2{h}{i}"
                    )
                    b2_ps = ps.tile([P, MT, M], F32, tag="kvit", bufs=3, name="iterps")
                    for a in range(MT):
                        for t in range(MT):
                            nc.tensor.matmul(
                                b2_ps[:, a, :],
                                B1s[h][:, t, a * P : (a + 1) * P],
                                V_sb[h][:, t, :],
                                start=(t == 0),
                                stop=(t == MT - 1),
                            )
                    nc.vector.tensor_copy(B2s[h][:], b2_ps[:])
                    # off-chain: tmp2 = -7/15*B2s + tmp (Pool)
                    tmp2 = wpool.tile(
                        [P, MT, M], F32, tag=f"wtmp2{h}", bufs=1, name=f"wtmp2{h}"
                    )
                    nc.vector.scalar_tensor_tensor(
                        tmp2[:],
                        B2s[h][:].bitcast(F32),
                        -7.0 / 15.0,
                        tmp[:],
                        ALU.mult,
                        ALU.add,
                    )
                    W_new = ipool.tile(
                        [P, MT, M], F32R, tag=f"Wn{h}", bufs=2, name=f"Wn{h}{i}"
                    )
                    b3_ps = ps.tile([P, MT, M], F32, tag="kvit", bufs=3, name="iterps3")
                    for a in range(MT):
                        for t in range(MT):
                            nc.tensor.matmul(
                                b3_ps[:, a, :],
                                B2s[h][:, t, a * P : (a + 1) * P],
                                V_sb[h][:, t, :],
                                start=(t == 0),
                                stop=(t == MT - 1),
                            )
                    # on-chain: W' = (1/15)*B3_psum + tmp2 (DVE)
                    nc.vector.scalar_tensor_tensor(
                        W_new[:],
                        b3_ps[:],
                        1.0 / 15.0,
                        tmp2[:],
                        ALU.mult,
                        ALU.add,
                    )
                    W_cur[h] = W_new

            # B of the folded 6th step: V5 = G @ W5 (per head)
            V5 = {}
            for h in range(2):
                V5[h] = ipool.tile(
                    [P, MT, M], F32R, tag=f"V{h}", bufs=1, name=f"V5{h}"
                )
                v5_ps = ps.tile([P, MT, M], F32, tag="kvit", bufs=3, name="iterps")
                for a in range(MT):
                    for t in range(MT):
                        nc.tensor.matmul(
                            v5_ps[:, a, :],
                            G_sb[h][:, t, a * P : (a + 1) * P],
                            W_cur[h][:, t, :],
                            start=(t == 0),
                            stop=(t == MT - 1),
                        )
                nc.vector.tensor_copy(V5[h][:], v5_ps[:])

            # ---------------- h1 tail ------------------------------------
            av_transposes(1, 0)
            av_transposes(1, 1)
            av_normalize(1)
            t1_h1 = t1_of(1)
            t1_all = [t1_h0, t1_h1]

            # ------- t2 = W t1 ; ohT = t2^T E1T (pre-normalized) ; y -------
            # t2 = W6^T t1 with the 6th iteration folded:
            # u0 = W5^T t1, u_{k+1} = B5^T u_k (B5 = V5), and
            # t2 = 3.25 u0 - 3.75 u1 + 1.75 u2 - 0.25 u3
            t2_all = []
            for h in range(2):
                us = []
                prev = t1_all[h]
                for k in range(4):
                    lhs = W_cur[h] if k == 0 else V5[h]
                    uk = wpool.tile(
                        [P, MT, DH], F32R, tag=f"u{h}", bufs=4, name=f"u{h}{k}"
                    )
                    for mc in range(MT):
                        u_ps = ps.tile([P, DH], F32, tag="small", bufs=2, name="ups")
                        for t in range(MT):
                            nc.tensor.matmul(
                                u_ps[:],
                                lhs[:, t, mc * P : (mc + 1) * P],
                                prev[:, t, :],
                                start=(t == 0),
                                stop=(t == MT - 1),
                            )
                        nc.vector.tensor_copy(uk[:, mc, :], u_ps[:])
                    us.append(uk)
                    prev = uk
                t2_sb = wpool.tile([P, MT, DH], F32R, tag=f"t2_{h}", bufs=1,
                                   name=f"t2_{h}")
                ta = wpool.tile([P, MT, DH], F32, tag=f"ta{h}", bufs=1,
                                name=f"ta{h}")
                nc.vector.scalar_tensor_tensor(
                    ta[:], us[1][:].bitcast(F32), -15.0 / 13.0,
                    us[0][:].bitcast(F32), ALU.mult, ALU.add,
                )
                tb = wpool.tile([P, MT, DH], F32, tag=f"tb{h}", bufs=1,
                                name=f"tb{h}")
                nc.vector.scalar_tensor_tensor(
                    tb[:], us[2][:].bitcast(F32), 7.0 / 13.0, ta[:],
                    ALU.mult, ALU.add,
                )
                tc2 = wpool.tile([P, MT, DH], F32, tag=f"tc{h}", bufs=1,
                                 name=f"tc{h}")
                nc.vector.scalar_tensor_tensor(
                    tc2[:], us[3][:].bitcast(F32), -1.0 / 13.0, tb[:],
                    ALU.mult, ALU.add,
                )
                nc.vector.tensor_scalar_mul(t2_sb[:], tc2[:], 3.25)
                t2_all.append(t2_sb)
            # ohT[hd, i] = sum_m t2[m, hd] * E1T[m, i]; heads stacked on
            # partitions 0:64 / 64:128 of one psum tile
            ohT2_sb = cpool.tile([P, NIH, 512], F32R, tag="ohT2", name="ohT2")
            for ih in range(NIH):
                for h in range(2):
                    oT_ps = ps.tile([DH, 512], F32, tag="s3", bufs=2, name="oTps")
                    for mc in range(MT):
                        nc.tensor.matmul(
                            oT_ps[:],
                            t2_all[h][:, mc, :],
                            E1T_sb[h][:, mc, ih * 512 : (ih + 1) * 512],
                            start=(mc == 0),
                            stop=(mc == MT - 1),
                        )
                    if h % 2 == 0:
                        nc.scalar.copy(
                            ohT2_sb[h * DH : (h + 1) * DH, ih, :], oT_ps[:]
                        )
                    else:
                        nc.vector.tensor_copy(
                            ohT2_sb[h * DH : (h + 1) * DH, ih, :], oT_ps[:]
                        )
            yr2 = y_d.rearrange("(t2 t p) f -> t2 p t f", t=2, p=P)
            for pair in range(NIT // 2):
                y_sb = wpool.tile([P, 2, DIM], F32, tag="ysb", bufs=2, name="ysb")
                for half in range(2):
                    it = pair * 2 + half
                    ih, sub = divmod(it, 4)
                    y_ps = ps.tile([P, DIM], F32, tag="kvit", bufs=3, name="yps")
                    nc.tensor.matmul(
                        y_ps[:],
                        ohT2_sb[:, ih, sub * P : (sub + 1) * P],
                        wout_sb[:],
                        start=True,
                        stop=True,
                    )
                    if half % 2 == 0:
                        nc.scalar.copy(y_sb[:, half, :], y_ps[:])
                    else:
                        nc.vector.tensor_copy(y_sb[:, half, :], y_ps[:])
                nc.sync.dma_start(yr2[pair], y_sb[:])

    _install_wait_split_hook(nc)
    return nc


_NC_CACHE = {}


def _get_nc():
    if "nc" not in _NC_CACHE:
        _NC_CACHE["nc"] = build_kernel()
    return _NC_CACHE["nc"]


def _make_in_maps(inputs):
    x = np.asarray(inputs["x"], np.float32)
    q_input = np.asarray(inputs["q_input"], np.float32)
    W_kv = np.asarray(inputs["W_kv"], np.float32)
    W_q = np.asarray(inputs["W_q"], np.float32)
    W_out = np.asarray(inputs["W_out"], np.float32)
    scale = np.float32(DH**-0.5)
    in_maps = []
    for core in range(NCORES):
        bi, g = divmod(core, 4)
        cs = slice(g * P, (g + 1) * P)
        in_maps.append(
            {
                "xT": np.ascontiguousarray(x[bi].T).astype(ml_dtypes.bfloat16),
                "qT_in": np.ascontiguousarray(q_input[bi].T).astype(ml_dtypes.bfloat16),
                "wq": np.ascontiguousarray(W_q[:, cs] * scale).astype(ml_dtypes.bfloat16),
                "wk": np.ascontiguousarray(W_kv[:, cs]).astype(ml_dtypes.bfloat16),
                "wv": np.ascontiguousarray(
                    W_kv[:, 512 + g * P : 512 + (g + 1) * P]
                ).astype(ml_dtypes.bfloat16),
                "wout": np.ascontiguousarray(W_out[cs, :]),
            }
        )
    return in_maps


def kernel(**inputs) -> np.ndarray:
    in_maps = _make_in_maps(inputs)
    nc = _get_nc()
    res = run_bass_kernel_spmd(nc, in_maps, core_ids=list(range(NCORES)))

    b_out = np.asarray(inputs["b_out"], np.float32)
    out = np.zeros((2, NQ, DIM), np.float32)
    for core in range(NCORES):
        out[core // 4] += res.results[core]["y"]
    out += b_out
    return out


# revision 41
# speedup vs baseline: 1.0014x; 1.0014x over previous
"""Nystromformer sparse attention on 8 Trainium2 NeuronCores.

Sharding: core = bi*4 + g handles batch bi (of 2) and heads {2g, 2g+1}
(of 8). All landmark/pinv work is per-(b,h); the final to_out matmul is
computed per-core against the matching W_out row-slice and the partial
(1024, 512) outputs are summed on the host (4 partials per batch).

Key algorithmic reformulation (exact algebra): the Moore-Penrose
iteration on attn2 (1024x256) runs entirely in 256x256 space:
z_k = W_k @ attn2^T with W symmetric, so each iteration is 4 256^3
matmuls instead of two 1024^3 ones. The global max(col)*max(row) init
scale is reproduced exactly: max(col)=1 (softmax rows), and max(row) is
an in-kernel AllReduce(max) across all 8 cores. The iteration starts
from W0 = (1/c) I and runs on the raw G (same trajectory as the
reference's z0 = attn2^T / c), so only W0 waits on the collective.

Precision: ALL wide matmuls run in float32r (HW probe: f32r rounds
operands to 12-bit mantissa; an emulator of the full pipeline predicts
~5e-3 max rel err for all-f32r vs ~4.5e-3 for the old fp32-mixed
schedule -- error is dominated by the always-f32r early Newton-Schulz
iterations, so fp32 elsewhere bought nothing at 4x PE cost).

Schedule (every engine queue is in-order, so emission order must track
readiness): the sim3->exp->av stream starts as soon as the first kT
slice lands (~6us) on a dedicated 3-buf PSUM tag; the landmark path
(s1, column sums via r1r-weighted matmuls of the unnormalized exp, and
the AllReduce) is emitted between av-groups 0 and 1 so the collective
launches ~35us; the two heads' Newton-Schulz chains are emitted
iteration-interleaved and reuse the projection-phase PSUM bufs; the h0
tail (transpose/normalize/t1) is emitted before the iterations, the h1
tail after, so no in-order queue head-blocks. Column-sum trick:
cs = sum_i E1[i,m]/r1[i] as an accumulated matmul with lhsT = 1/r1,
removing the A-normalization from the collective's critical path.
"""

import json
import sys

for _p in ("/opt/trn_rl_repo", "/root/.axon_site/_ro/trn_rl_repo"):
    if _p not in sys.path:
        sys.path.append(_p)

import ml_dtypes
import numpy as np

import concourse.bass as bass
import concourse.mybir as mybir
import concourse.tile as tile
from concourse.bass_utils import run_bass_kernel_spmd

F32 = mybir.dt.float32
F32R = mybir.dt.float32r
BF16 = mybir.dt.bfloat16
AX = mybir.AxisListType
ALU = mybir.AluOpType
EXP = mybir.ActivationFunctionType.Exp

P = 128
DIM = 512
CH = 4  # contraction chunks of 128 over DIM
N = 4096
NS = 8  # 512-wide n slices
NJ = 32  # 128-wide j tiles
NQ = 1024
NIH = 2  # 512-wide i halves
NIT = 8  # 128-wide i tiles
M = 256
MT = 2  # 128-wide m tiles
DH = 64
ITERS = 6
NCORES = 8


def _split_multi_waits(bir_json_bytes: bytes) -> bytes:
    """Walrus in this container accepts one sync wait per instruction."""
    bir = json.loads(bir_json_bytes)
    for fn in bir.get("functions", []):
        for blk in fn.get("blocks", []):
            out = []
            for inst in blk.get("instructions", []):
                si = inst.get("sync_info")
                waits = (si or {}).get("on_wait") or []
                if len(waits) > 1:
                    for i, w in enumerate(waits[:-1]):
                        out.append(
                            {
                                "name": f"{inst['name']}-wsplit{i}",
                                "opcode": "NoOp",
                                "engine": inst["engine"],
                                "ins": [],
                                "outs": [],
                                "sync_info": {"on_wait": [w], "on_update": []},
                            }
                        )
                    si["on_wait"] = [waits[-1]]
                out.append(inst)
            blk["instructions"] = out
    return json.dumps(bir).encode()


def _install_wait_split_hook(nc):
    orig = nc.to_json_bytes

    def patched():
        return _split_multi_waits(orig())

    nc.to_json_bytes = patched


def _diag_ones(nc, ap, k):
    nc.gpsimd.affine_select(
        out=ap,
        in_=ap,
        compare_op=ALU.not_equal,
        fill=1.0,
        base=0,
        pattern=[[-1, k]],
        channel_multiplier=1,
    )


def build_kernel() -> bass.Bass:
    nc = bass.Bass("TRN2", num_devices=NCORES)

    xT_d = nc.dram_tensor("xT", [DIM, N], BF16, kind="ExternalInput")
    qT_d = nc.dram_tensor("qT_in", [DIM, NQ], BF16, kind="ExternalInput")
    wq_d = nc.dram_tensor("wq", [DIM, P], BF16, kind="ExternalInput")
    wk_d = nc.dram_tensor("wk", [DIM, P], BF16, kind="ExternalInput")
    wv_d = nc.dram_tensor("wv", [DIM, P], BF16, kind="ExternalInput")
    wout_d = nc.dram_tensor("wout", [P, DIM], F32R, kind="ExternalInput")
    y_d = nc.dram_tensor("y", [NQ, DIM], F32, kind="ExternalOutput")

    xr = xT_d.rearrange("(c p) n -> c p n", p=P)
    qr = qT_d.rearrange("(c p) n -> c p n", p=P)
    yr = y_d.rearrange("(t p) f -> t p f", p=P)

    with tile.TileContext(nc) as tc:
        with (
            tc.tile_pool(name="const", bufs=1) as cpool,
            tc.tile_pool(name="work", bufs=3) as wpool,
            tc.tile_pool(name="iter", bufs=2) as ipool,
            tc.tile_pool(name="ps", bufs=1, space="PSUM") as ps,
            tc.tile_pool(name="dram", bufs=1, space="DRAM") as dpool,
        ):
            # ------------- weights / constants (q-path DMAs on ACT queue) --
            wq_sb = cpool.tile([P, CH, P], BF16, tag="wq", name="wq")
            wk_sb = cpool.tile([P, CH, P], BF16, tag="wk", name="wk")
            wv_sb = cpool.tile([P, CH, P], BF16, tag="wv", name="wv")
            wout_sb = cpool.tile([P, DIM], F32R, tag="wout", name="wout")
            nc.sync.dma_start(wq_sb[:], wq_d.rearrange("(c p) m -> p c m", p=P))
            nc.sync.dma_start(wk_sb[:], wk_d.rearrange("(c p) m -> p c m", p=P))
            nc.sync.dma_start(wv_sb[:], wv_d.rearrange("(c p) m -> p c m", p=P))

            ones_col = cpool.tile([P, 1], F32, tag="ones", name="ones")
            nc.vector.memset(ones_col[:], 1.0)
            ident = cpool.tile([P, P], F32, tag="ident", name="ident")
            nc.vector.memset(ident[:], 0.0)
            _diag_ones(nc, ident[:], P)
            ident2 = cpool.tile([P, MT, M], F32, tag="ident2", name="ident2")
            nc.vector.memset(ident2[:], 0.0)
            _diag_ones(nc, ident2[:, 0, 0:P], P)
            _diag_ones(nc, ident2[:, 1, P:M], P)

            # ---------------- q projection (f32r) -------------------------
            qT_sb = cpool.tile([P, NQ], F32R, tag="qT", name="qT")
            q_pss = [
                ps.tile([P, 512], F32, tag="kvit", bufs=3, name="qps")
                for _ in range(NIH)
            ]
            for c in range(CH):
                qb = wpool.tile([P, NQ], BF16, tag="qb", bufs=3, name="qb")
                nc.sync.dma_start(qb[:], qr[c])
                for ih in range(NIH):
                    nc.tensor.matmul(
                        q_pss[ih][:],
                        wq_sb[:, c, :],
                        qb[:, ih * 512 : (ih + 1) * 512],
                        start=(c == 0),
                        stop=(c == CH - 1),
                    )
            for ih in range(NIH):
                nc.vector.tensor_copy(
                    qT_sb[:, ih * 512 : (ih + 1) * 512], q_pss[ih][:]
                )
            nc.sync.dma_start(wout_sb[:], wout_d[:])

            # ------- k/v projections + incremental landmark pool + vtr ----
            # vaug layout [P(j), NJ, 130]: 0:64 h0 v, 64 ones, 65:129 h1 v,
            # 129 ones
            kT_sb = cpool.tile([P, N], F32R, tag="kT", name="kT")
            vT_sb = cpool.tile([P, N], F32R, tag="vT", name="vT")
            klT_sb = cpool.tile([P, M], F32R, tag="klT", name="klT")
            vaug_sb = cpool.tile([P, NJ, 130], BF16, tag="vaug", name="vaug")
            for h in range(2):
                nc.vector.tensor_copy(
                    vaug_sb[:, :, h * 65 + DH : h * 65 + DH + 1],
                    ones_col[:, 0:1, None].to_broadcast((P, NJ, 1)),
                )
            for half in range(2):
                hsl = slice(half * 2048, (half + 1) * 2048)
                xbs = []
                for c in range(CH):
                    xb = wpool.tile([P, 2048], BF16, tag="xb", bufs=5, name="xb")
                    nc.sync.dma_start(xb[:], xr[c][:, hsl])
                    xbs.append(xb)
                for nsl in range(4):
                    ns = half * 4 + nsl
                    sl = slice(ns * 512, (ns + 1) * 512)
                    ssl = slice(nsl * 512, (nsl + 1) * 512)
                    k_ps = ps.tile([P, 512], F32, tag="kvit", bufs=3, name="kps")
                    v_ps = ps.tile([P, 512], F32, tag="kvit", bufs=3, name="vps")
                    for c in range(CH):
                        nc.tensor.matmul(
                            k_ps[:],
                            wk_sb[:, c, :],
                            xbs[c][:, ssl],
                            start=(c == 0),
                            stop=(c == CH - 1),
                        )
                        nc.tensor.matmul(
                            v_ps[:],
                            wv_sb[:, c, :],
                            xbs[c][:, ssl],
                            start=(c == 0),
                            stop=(c == CH - 1),
                        )
                    nc.vector.tensor_copy(kT_sb[:, sl], k_ps[:])
                    nc.vector.tensor_copy(vT_sb[:, sl], v_ps[:])
                    with nc.allow_low_precision(
                        reason="f32r tiles carry fp32 bits"
                    ):
                        nc.vector.reduce_sum(
                            klT_sb[:, ns * 32 : (ns + 1) * 32],
                            kT_sb[:, sl]
                            .bitcast(F32)
                            .rearrange("p (m l) -> p m l", l=16),
                            axis=AX.X,
                        )
                    for sub in range(4):
                        jt = ns * 4 + sub
                        tr_ps = ps.tile([P, P], F32, tag="small", bufs=2, name="vtrps")
                        nc.tensor.transpose(
                            tr_ps[:],
                            vT_sb[:, jt * P : (jt + 1) * P].bitcast(F32),
                            ident[:],
                        )
                        nc.vector.tensor_copy(
                            vaug_sb[:, jt, :].rearrange("p (g d) -> p g d", g=2)[
                                :, :, 0:DH
                            ],
                            tr_ps[:].rearrange("p (g d) -> p g d", g=2),
                        )

            # helpers for the s3 -> exp -> av stream -----------------------
            avu_sb = [
                cpool.tile([P, NIH, 512], F32, tag=f"avu{h}", name=f"avu{h}")
                for h in range(2)
            ]

            def av_group(h, ih):
                hs = slice(h * DH, (h + 1) * DH)
                av_ps = ps.tile([DH + 1, 512], F32, tag="hold", bufs=1, name="avps")
                for jt in range(NJ):
                    s3_ps = ps.tile([P, 512], F32, tag="s3", bufs=2, name="s3ps")
                    nc.tensor.matmul(
                        s3_ps[:],
                        kT_sb[hs, jt * P : (jt + 1) * P],
                        qT_sb[hs, ih * 512 : (ih + 1) * 512],
                        start=True,
                        stop=True,
                    )
                    e3 = wpool.tile([P, 512], BF16, tag="e3", name="e3")
                    nc.scalar.activation(e3[:], s3_ps[:], EXP)
                    nc.tensor.matmul(
                        av_ps[:],
                        vaug_sb[:, jt, h * 65 : h * 65 + 65],
                        e3[:],
                        start=(jt == 0),
                        stop=(jt == NJ - 1),
                    )
                # ACT copies the accumulator out: keeps DVE queue untangled
                nc.scalar.copy(avu_sb[h][: DH + 1, ih, :], av_ps[:])

            at_sb = [
                cpool.tile([P, NIT, DH + 1], F32, tag=f"at{h}", name=f"at{h}")
                for h in range(2)
            ]

            def av_transposes(h, ih):
                for isub in range(4):
                    it = ih * 4 + isub
                    at_ps = ps.tile([P, DH + 1], F32, tag="small", bufs=2, name="avtps")
                    nc.tensor.transpose(
                        at_ps[:],
                        avu_sb[h][: DH + 1, ih, isub * P : (isub + 1) * P],
                        ident[: DH + 1, : DH + 1],
                    )
                    nc.vector.tensor_copy(at_sb[h][:, it, :], at_ps[:])

            av_sb = [
                cpool.tile([P, NIT, DH], F32R, tag=f"av{h}", name=f"av{h}")
                for h in range(2)
            ]
            r3c_sb = [
                cpool.tile([P, NIT], F32, tag=f"r3c{h}", name=f"r3c{h}")
                for h in range(2)
            ]
            r3r_sb = [
                cpool.tile([P, NIT], F32, tag=f"r3r{h}", name=f"r3r{h}")
                for h in range(2)
            ]

            def av_normalize(h):
                for it in range(NIT):
                    nc.vector.tensor_copy(
                        r3c_sb[h][:, it : it + 1], at_sb[h][:, it, DH : DH + 1]
                    )
                nc.vector.reciprocal(r3r_sb[h][:], r3c_sb[h][:])
                for it in range(NIT):
                    nc.vector.tensor_scalar_mul(
                        av_sb[h][:, it, :],
                        at_sb[h][:, it, 0:DH],
                        r3r_sb[h][:, it : it + 1],
                    )

            A_sb = [
                cpool.tile([P, NIT, M], F32R, tag=f"A{h}", name=f"A{h}") for h in range(2)
            ]

            def t1_of(h):
                t1_sb = wpool.tile([P, MT, DH], F32R, tag=f"t1_{h}", bufs=1,
                                   name=f"t1_{h}")
                for mc in range(MT):
                    t1_ps = ps.tile([P, DH], F32, tag="small", bufs=2, name="t1ps")
                    for it in range(NIT):
                        nc.tensor.matmul(
                            t1_ps[:],
                            A_sb[h][:, it, mc * P : (mc + 1) * P],
                            av_sb[h][:, it, :],
                            start=(it == 0),
                            stop=(it == NIT - 1),
                        )
                    nc.vector.tensor_copy(t1_sb[:, mc, :], t1_ps[:])
                return t1_sb

            # ---------------- landmark path: s1 -> cs -> AllReduce --------
            r1s_sb = [
                cpool.tile([P, NIT], F32, tag=f"r1s{h}", name=f"r1s{h}")
                for h in range(2)
            ]
            r1r_sb = [
                cpool.tile([P, NIT], F32R, tag=f"r1r{h}", name=f"r1r{h}")
                for h in range(2)
            ]
            cmax2_sb = wpool.tile([1, 2], F32, tag="cmax2", name="cmax2")
            for h in range(2):
                hs = slice(h * DH, (h + 1) * DH)
                for it in range(NIT):
                    s1_ps = ps.tile([P, M], F32, tag="small", bufs=2, name="s1ps")
                    nc.tensor.matmul(
                        s1_ps[:],
                        qT_sb[hs, it * P : (it + 1) * P],
                        klT_sb[hs, :],
                        start=True,
                        stop=True,
                    )
                    # A_sb holds UNNORMALIZED exp until the cs matmuls read it
                    nc.scalar.activation(
                        A_sb[h][:, it, :],
                        s1_ps[:],
                        EXP,
                        accum_out=r1s_sb[h][:, it : it + 1],
                    )
                with nc.allow_low_precision(reason="f32r carries fp32 bits"):
                    nc.vector.reciprocal(r1r_sb[h][:], r1s_sb[h][:])
                # cs[m] = sum_i E1[i,m]/r1[i]: lhsT = 1/r1 column per i-tile
                cs_ps = ps.tile([1, M], F32, tag="small", bufs=2, name="csps")
                for it in range(NIT):
                    nc.tensor.matmul(
                        cs_ps[:],
                        r1r_sb[h][:, it : it + 1],
                        A_sb[h][:, it, :],
                        start=(it == 0),
                        stop=(it == NIT - 1),
                    )
                nc.vector.reduce_max(cmax2_sb[0:1, h : h + 1], cs_ps[:], axis=AX.X)
            cmax_sb = wpool.tile([1, 1], F32, tag="cmax", name="cmax")
            nc.vector.reduce_max(cmax_sb[:], cmax2_sb[:], axis=AX.X)
            bounce_sb = wpool.tile([1, 16], F32, tag="bounce", name="bounce")
            nc.vector.tensor_copy(bounce_sb[:], cmax_sb[0:1, 0:1].to_broadcast((1, 16)))
            cin_dram = dpool.tile([1, 16], F32)
            cout_dram = dpool.tile([1, 16 * NCORES], F32)
            nc.sync.dma_start(cin_dram[:], bounce_sb[:])
            nc.gpsimd.collective_compute(
                "AllGather",
                ALU.bypass,
                replica_groups=[list(range(NCORES))],
                ins=[cin_dram.opt()],
                outs=[cout_dram.opt()],
            )
            c128_sb = wpool.tile([P, 16 * NCORES], F32, tag="c128", name="c128")
            nc.sync.dma_start(
                c128_sb[:], cout_dram[0:1, :].to_broadcast((P, 16 * NCORES))
            )

            # A normalize (off critical path) + G = A^T A ------------------
            for h in range(2):
                for it in range(NIT):
                    nc.vector.tensor_scalar_mul(
                        A_sb[h][:, it, :],
                        A_sb[h][:, it, :].bitcast(F32),
                        r1r_sb[h][:, it : it + 1].bitcast(F32),
                    )
            # E1T = transpose of NORMALIZED A: the 1/r1 row scaling rides
            # along, so the final E1 @ t2 needs no normalization.
            E1T_sb = [
                cpool.tile([P, MT, NQ], F32R, tag=f"E1T{h}", name=f"E1T{h}")
                for h in range(2)
            ]
            for h in range(2):
                for it in range(NIT):
                    et_ps = ps.tile([P, P], F32, tag="small", bufs=2, name="etps")
                    for mt in range(MT):
                        nc.tensor.transpose(
                            et_ps[:],
                            A_sb[h][:, it, mt * P : (mt + 1) * P].bitcast(F32),
                            ident[:],
                        )
                        nc.vector.tensor_copy(
                            E1T_sb[h][:, mt, it * P : (it + 1) * P], et_ps[:]
                        )
            G_sb = [
                cpool.tile([P, MT, M], F32R, tag=f"G{h}", name=f"G{h}") for h in range(2)
            ]
            for h in range(2):
                for mc in range(MT):
                    g_ps = ps.tile([P, M], F32, tag="small", bufs=2, name="gps")
                    for it in range(NIT):
                        nc.tensor.matmul(
                            g_ps[:],
                            A_sb[h][:, it, mc * P : (mc + 1) * P],
                            A_sb[h][:, it, :],
                            start=(it == 0),
                            stop=(it == NIT - 1),
                        )
                    nc.vector.tensor_copy(G_sb[h][:, mc, :], g_ps[:])

            # ---------------- av group 0 (h0, ih0) ------------------------
            av_group(0, 0)
            av_transposes(0, 0)

            # ---------------- av group 1 + h0 tail ------------------------
            av_group(0, 1)
            av_transposes(0, 1)
            av_normalize(0)
            t1_h0 = t1_of(0)

            # ---------------- av groups 2, 3 (h1) -------------------------
            av_group(1, 0)
            av_group(1, 1)

            # ---------------- rc, W0, Newton-Schulz (head-interleaved) ----
            cmax128_sb = wpool.tile([P, 1], F32, tag="cmax128", name="cmax128")
            nc.vector.reduce_max(cmax128_sb[:], c128_sb[:], axis=AX.X)
            rc_sb = cpool.tile([P, 1], F32, tag="rc", name="rc")
            nc.vector.reciprocal(rc_sb[:], cmax128_sb[:])
            W_cur = []
            for h in range(2):
                W0 = ipool.tile([P, MT, M], F32R, tag=f"W{h}", bufs=1, name=f"W{h}")
                nc.vector.tensor_scalar_mul(W0[:], ident2[:], rc_sb[:, 0:1])
                W_cur.append(W0)
            for i in range(ITERS - 1):
                V_sb, B1s, B2s = {}, {}, {}
                for h in range(2):
                    V_sb[h] = ipool.tile(
                        [P, MT, M], F32R, tag=f"V{h}", bufs=1, name=f"V{h}{i}"
                    )
                    v_ps2 = ps.tile([P, MT, M], F32, tag="kvit", bufs=3, name="iterps")
                    for a in range(MT):
                        for t in range(MT):
                            nc.tensor.matmul(
                                v_ps2[:, a, :],
                                G_sb[h][:, t, a * P : (a + 1) * P],
                                W_cur[h][:, t, :],
                                start=(t == 0),
                                stop=(t == MT - 1),
                            )
                    nc.vector.tensor_copy(V_sb[h][:], v_ps2[:])
                for h in range(2):
                    B1s[h] = ipool.tile(
                        [P, MT, M], F32R, tag=f"B1{h}", bufs=1, name=f"B1{h}{i}"
                    )
                    b1_ps = ps.tile([P, MT, M], F32, tag="kvit", bufs=3, name="iterps")
                    for a in range(MT):
                        for t in range(MT):
                            nc.tensor.matmul(
                                b1_ps[:, a, :],
                                W_cur[h][:, t, a * P : (a + 1) * P],
                                V_sb[h][:, t, :],
                                start=(t == 0),
                                stop=(t == MT - 1),
                            )
                    nc.vector.tensor_scalar_mul(B1s[h][:], b1_ps[:], -3.75)
                for h in range(2):
                    # off-chain: tmp = 3.25*W + B1s (Pool, SBUF-only)
                    tmp = wpool.tile(
                        [P, MT, M], F32, tag=f"wtmp{h}", bufs=1, name=f"wtmp{h}"
                    )
                    nc.vector.scalar_tensor_tensor(
                        tmp[:],
                        W_cur[h][:].bitcast(F32),
                        3.25,
                        B1s[h][:].bitcast(F32),
                        ALU.mult,
                        ALU.add,
                    )
                    B2s[h] = ipool.tile(
                        [P, MT, M], F32R, tag=f"B2{h}", bufs=1, name=f"B2{h}{i}"
                    )
                    b2_ps = ps.tile([P, MT, M], F32, tag="kvit", bufs=3, name="iterps")
                    for a in range(MT):
                        for t in range(MT):
                            nc.tensor.matmul(
                                b2_ps[:, a, :],
                                B1s[h][:, t, a * P : (a + 1) * P],
                                V_sb[h][:, t, :],
                                start=(t == 0),
                                stop=(t == MT - 1),
                            )
                    nc.vector.tensor_copy(B2s[h][:], b2_ps[:])
                    # off-chain: tmp2 = -7/15*B2s + tmp (Pool)
                    tmp2 = wpool.tile(
                        [P, MT, M], F32, tag=f"wtmp2{h}", bufs=1, name=f"wtmp2{h}"
                    )
                    nc.vector.scalar_tensor_tensor(
                        tmp2[:],
                        B2s[h][:].bitcast(F32),
                        -7.0 / 15.0,
                        tmp[:],
                        ALU.mult,
                        ALU.add,
                    )
                    W_new = ipool.tile(
                        [P, MT, M], F32R, tag=f"Wn{h}", bufs=2, name=f"Wn{h}{i}"
                    )
                    b3_ps = ps.tile([P, MT, M], F32, tag="kvit", bufs=3, name="iterps3")
                    for a in range(MT):
                        for t in range(MT):
                            nc.tensor.matmul(
                                b3_ps[:, a, :],
                                B2s[h][:, t, a * P : (a + 1) * P],
                                V_sb[h][:, t, :],
                                start=(t == 0),
                                stop=(t == MT - 1),
                            )
                    # on-chain: W' = (1/15)*B3_psum + tmp2 (DVE)
                    nc.vector.scalar_tensor_tensor(
                        W_new[:],
                        b3_ps[:],
                        1.0 / 15.0,
                        tmp2[:],
                        ALU.mult,
                        ALU.add,
                    )
                    W_cur[h] = W_new

            # B of the folded 6th step: V5 = G @ W5 (per head)
            V5 = {}
            for h in range(2):
                V5[h] = ipool.tile(
                    [P, MT, M], F32R, tag=f"V{h}", bufs=1, name=f"V5{h}"
                )
                v5_ps = ps.tile([P, MT, M], F32, tag="kvit", bufs=3, name="iterps")
                for a in range(MT):
                    for t in range(MT):
                        nc.tensor.matmul(
                            v5_ps[:, a, :],
                            G_sb[h][:, t, a * P : (a + 1) * P],
                            W_cur[h][:, t, :],
                            start=(t == 0),
                            stop=(t == MT - 1),
                        )
                nc.vector.tensor_copy(V5[h][:], v5_ps[:])

            # ---------------- h1 tail ------------------------------------
            av_transposes(1, 0)
            av_transposes(1, 1)
            av_normalize(1)
            t1_h1 = t1_of(1)
            t1_all = [t1_h0, t1_h1]

            # ------- t2 = W t1 ; ohT = t2^T E1T (pre-normalized) ; y -------
            # t2 = W6^T t1 with the 6th iteration folded:
            # u0 = W5^T t1, u_{k+1} = B5^T u_k (B5 = V5), and
            # t2 = 3.25 u0 - 3.75 u1 + 1.75 u2 - 0.25 u3
            t2_all = []
            for h in range(2):
                us = []
                prev = t1_all[h]
                for k in range(4):
                    lhs = W_cur[h] if k == 0 else V5[h]
                    uk = wpool.tile(
                        [P, MT, DH], F32R, tag=f"u{h}", bufs=4, name=f"u{h}{k}"
                    )
                    for mc in range(MT):
                        u_ps = ps.tile([P, DH], F32, tag="small", bufs=2, name="ups")
                        for t in range(MT):
                            nc.tensor.matmul(
                                u_ps[:],
                                lhs[:, t, mc * P : (mc + 1) * P],
                                prev[:, t, :],
                                start=(t == 0),
                                stop=(t == MT - 1),
                            )
                        if k == 0:
                            nc.vector.tensor_scalar_mul(
                                uk[:, mc, :], u_ps[:], 3.25
                            )
                        else:
                            nc.vector.tensor_copy(uk[:, mc, :], u_ps[:])
                    us.append(uk)
                    prev = uk
                t2_sb = wpool.tile([P, MT, DH], F32R, tag=f"t2_{h}", bufs=1,
                                   name=f"t2_{h}")
                # u0 is pre-scaled by 3.25; u1 = B^T(3.25 u0) = 3.25*u1_raw,
                # so t2 = u0' - (15/13)u1' + (7/13)*3.25*u2 - (1/13)*3.25*u3
                ta = wpool.tile([P, MT, DH], F32, tag=f"ta{h}", bufs=1,
                                name=f"ta{h}")
                nc.vector.scalar_tensor_tensor(
                    ta[:], us[1][:].bitcast(F32), -15.0 / 13.0,
                    us[0][:].bitcast(F32), ALU.mult, ALU.add,
                )
                tb = wpool.tile([P, MT, DH], F32, tag=f"tb{h}", bufs=1,
                                name=f"tb{h}")
                nc.vector.scalar_tensor_tensor(
                    tb[:], us[2][:].bitcast(F32), 7.0 / 13.0, ta[:],
                    ALU.mult, ALU.add,
                )
                nc.vector.scalar_tensor_tensor(
                    t2_sb[:], us[3][:].bitcast(F32), -1.0 / 13.0, tb[:],
                    ALU.mult, ALU.add,
                )
                t2_all.append(t2_sb)
            # ohT[hd, i] = sum_m t2[m, hd] * E1T[m, i]; heads stacked on
            # partitions 0:64 / 64:128 of one psum tile
            ohT2_sb = cpool.tile([P, NIH, 512], F32R, tag="ohT2", name="ohT2")
            for ih in range(NIH):
                for h in range(2):
                    oT_ps = ps.tile([DH, 512], F32, tag="s3", bufs=2, name="oTps")
                    for mc in range(MT):
                        nc.tensor.matmul(
                            oT_ps[:],
                            t2_all[h][:, mc, :],
                            E1T_sb[h][:, mc, ih * 512 : (ih + 1) * 512],
                            start=(mc == 0),
                            stop=(mc == MT - 1),
                        )
                    if h % 2 == 0:
                        nc.scalar.copy(
                            ohT2_sb[h * DH : (h + 1) * DH, ih, :], oT_ps[:]
                        )
                    else:
                        nc.vector.tensor_copy(
                            ohT2_sb[h * DH : (h + 1) * DH, ih, :], oT_ps[:]
                        )
            yr2 = y_d.rearrange("(t2 t p) f -> t2 p t f", t=2, p=P)
            for pair in range(NIT // 2):
                y_sb = wpool.tile([P, 2, DIM], F32, tag="ysb", bufs=2, name="ysb")
                for half in range(2):
                    it = pair * 2 + half
                    ih, sub = divmod(it, 4)
                    y_ps = ps.tile([P, DIM], F32, tag="kvit", bufs=3, name="yps")
                    nc.tensor.matmul(
                        y_ps[:],
                        ohT2_sb[:, ih, sub * P : (sub + 1) * P],
                        wout_sb[:],
                        start=True,
                        stop=True,
                    )
                    if half % 2 == 0:
                        nc.scalar.copy(y_sb[:, half, :], y_ps[:])
                    else:
                        nc.vector.tensor_copy(y_sb[:, half, :], y_ps[:])
                nc.sync.dma_start(yr2[pair], y_sb[:])

    _install_wait_split_hook(nc)
    return nc


_NC_CACHE = {}


def _get_nc():
    if "nc" not in _NC_CACHE:
        _NC_CACHE["nc"] = build_kernel()
    return _NC_CACHE["nc"]


def _make_in_maps(inputs):
    x = np.asarray(inputs["x"], np.float32)
    q_input = np.asarray(inputs["q_input"], np.float32)
    W_kv = np.asarray(inputs["W_kv"], np.float32)
    W_q = np.asarray(inputs["W_q"], np.float32)
    W_out = np.asarray(inputs["W_out"], np.float32)
    scale = np.float32(DH**-0.5)
    in_maps = []
    for core in range(NCORES):
        bi, g = divmod(core, 4)
        cs = slice(g * P, (g + 1) * P)
        in_maps.append(
            {
                "xT": np.ascontiguousarray(x[bi].T).astype(ml_dtypes.bfloat16),
                "qT_in": np.ascontiguousarray(q_input[bi].T).astype(ml_dtypes.bfloat16),
                "wq": np.ascontiguousarray(W_q[:, cs] * scale).astype(ml_dtypes.bfloat16),
                "wk": np.ascontiguousarray(W_kv[:, cs]).astype(ml_dtypes.bfloat16),
                "wv": np.ascontiguousarray(
                    W_kv[:, 512 + g * P : 512 + (g + 1) * P]
                ).astype(ml_dtypes.bfloat16),
                "wout": np.ascontiguousarray(W_out[cs, :]),
            }
        )
    return in_maps


def kernel(**inputs) -> np.ndarray:
    in_maps = _make_in_maps(inputs)
    nc = _get_nc()
    res = run_bass_kernel_spmd(nc, in_maps, core_ids=list(range(NCORES)))

    b_out = np.asarray(inputs["b_out"], np.float32)
    out = np.zeros((2, NQ, DIM), np.float32)
    for core in range(NCORES):
        out[core // 4] += res.results[core]["y"]
    out += b_out
    return out
